# revision 1
# baseline (speedup 1.0000x reference)
"""Bass/Tile kernel for KeyFrameAttention on 8 NeuronCores (TRN2).

Math (per batch item b):
    q = x @ Wq + bq ; k = x @ Wk + bk ; v = x @ Wv + bv
    scores[n,m] = q[n]·k[m];  masked-fill(Mask==0, -1e20); softmax over m of scores/sqrt(C)
    att_feat[n,c] = sum_m v[m,c] * attn[m,n]          (attention applied TRANSPOSED)
    out = att_feat @ Wr + br

Sharding: data-parallel over batch B=64 -> 8 batch items per core.

Host-side prep inside kernel(): cast x / Mask / W* to bf16 (validated: end-to-end
rel err ~5e-3 vs fp32 reference thanks to fp32 PSUM accumulation).

Per-core plan (bf16 matmuls, fp32 PSUM accumulation):
  xT  [C,N]   via PE transposes of x tiles (contraction needs c on partitions)
  qT,kT [C,N] = W.T @ x.T   (lhsT = W tile, rhs = xT)      -> bf16 SBUF
  v   [N,C]   natural       (lhsT = xT tile, rhs = Wv)     -> bf16 SBUF
  scores tile [128n, 512m] = qT.T @ kT ; masked softmax via the (+BIG)*mask trick:
      t = (scores + BIG)*mask ; e = exp(s*t - s*max(t)) ; masked -> exp(-s*max) == 0
  att_featT [C,N]: lhsT = v tile, rhs = attn tile (no attn transpose needed)
  out [N,C]:  lhsT = afT tile, rhs = Wr ; + br ; DMA out.

Weights are streamed from HBM per batch item (SBUF can't hold 4x CxC + working set).
"""

import math

import numpy as np

B, N, C = 64, 512, 1280
NCORES = 8
BPC = B // NCORES  # batch items per core
P = 128
NT = N // P  # 4  n-tiles
CT = C // P  # 10 c-tiles
BIG = 10000.0
SCALE = 1.0 / math.sqrt(float(C))
CF_SLICES = [(0, 512), (512, 512), (1024, 256)]  # free-dim chunks of C

_CACHE = {}


def _build_nc():
    import concourse.bass as bass
    import concourse.mybir as mybir
    import concourse.tile as tile
    from concourse import bacc
    from concourse.masks import make_identity

    f32 = mybir.dt.float32
    bf16 = mybir.dt.bfloat16
    AF = mybir.ActivationFunctionType
    ALU = mybir.AluOpType

    # Bacc (not raw Bass): its finalize() runs move_matmul_waits_to_ldweights +
    # generate_event_semaphores, which split multi-sem waits that otherwise
    # exceed the per-instruction ISA wait-slot limit in walrus codegen.
    nc = bacc.Bacc(None, target_bir_lowering=False)
    x_h = nc.declare_dram_parameter("x", [BPC, N, C], bf16, isOutput=False)
    m_h = nc.declare_dram_parameter("mask", [BPC, N, N], bf16, isOutput=False)
    wq_h = nc.declare_dram_parameter("wq", [C, C], bf16, isOutput=False)
    bq_h = nc.declare_dram_parameter("bq", [C], f32, isOutput=False)
    wk_h = nc.declare_dram_parameter("wk", [C, C], bf16, isOutput=False)
    bk_h = nc.declare_dram_parameter("bk", [C], f32, isOutput=False)
    wv_h = nc.declare_dram_parameter("wv", [C, C], bf16, isOutput=False)
    bv_h = nc.declare_dram_parameter("bv", [C], f32, isOutput=False)
    wr_h = nc.declare_dram_parameter("wr", [C, C], bf16, isOutput=False)
    br_h = nc.declare_dram_parameter("br", [C], f32, isOutput=False)
    out_h = nc.declare_dram_parameter("out", [BPC, N, C], f32, isOutput=True)

    def bcast_ap(handle):
        ap0 = handle[:]
        return bass.AP(tensor=ap0.tensor, offset=ap0.offset, ap=[[0, P], ap0.ap[0]])

    with tile.TileContext(nc) as tc:
        with (
            tc.tile_pool(name="sb", bufs=1) as sb,
            tc.tile_pool(name="ps", bufs=1, space="PSUM") as ps,
        ):
            # ---- biases (one-time)
            bq_sb = sb.tile([P, CT], f32, tag="bq", bufs=1, name="bq_sb")
            nc.sync.dma_start(out=bq_sb, in_=bq_h[:].rearrange("(co p) -> p co", p=P))
            bk_sb = sb.tile([P, CT], f32, tag="bk", bufs=1, name="bk_sb")
            nc.sync.dma_start(out=bk_sb, in_=bk_h[:].rearrange("(co p) -> p co", p=P))
            bv_sb = sb.tile([P, C], f32, tag="bv", bufs=1, name="bv_sb")
            nc.sync.dma_start(out=bv_sb, in_=bcast_ap(bv_h))
            br_sb = sb.tile([P, C], f32, tag="br", bufs=1, name="br_sb")
            nc.sync.dma_start(out=br_sb, in_=bcast_ap(br_h))

            for b in range(BPC):
                # ---- Phase A: DMA-transpose x (bf16, 2-byte dtype) -> xT [c, n]
                xT = []
                for ct in range(CT):
                    xt = sb.tile([P, N], bf16, tag="xT", bufs=22, name=f"xt{b}_{ct}")
                    nc.sync.dma_start_transpose(
                        out=xt, in_=x_h[b, :, ct * P : (ct + 1) * P]
                    )
                    xT.append(xt)

                # ---- Phase B: qT, kT (lhsT = W tile), v (lhsT = xT tile)
                qT, kT = [], []
                for wh, dst, bias, wtag in (
                    (wq_h, qT, bq_sb, "q"),
                    (wk_h, kT, bk_sb, "k"),
                ):
                    wt = []
                    for ki in range(CT):
                        w = sb.tile(
                            [P, C], bf16, tag="w", bufs=16, name=f"w{b}_{wtag}_{ki}"
                        )
                        nc.sync.dma_start(out=w, in_=wh[ki * P : (ki + 1) * P, :])
                        wt.append(w)
                    for co in range(CT):
                        pm = ps.tile(
                            [P, N], f32, tag="mm", bufs=6, name=f"pq{b}_{wtag}_{co}"
                        )
                        for ki in range(CT):
                            nc.tensor.matmul(
                                pm,
                                wt[ki][:, co * P : (co + 1) * P],
                                xT[ki],
                                start=(ki == 0),
                                stop=(ki == CT - 1),
                            )
                        sbt = sb.tile(
                            [P, N], bf16, tag="qkT", bufs=22, name=f"qk{b}_{wtag}_{co}"
                        )
                        nc.vector.tensor_scalar_add(
                            out=sbt, in0=pm, scalar1=bias[:, co : co + 1]
                        )
                        dst.append(sbt)

                wv_t = []
                for ki in range(CT):
                    w = sb.tile([P, C], bf16, tag="w", bufs=16, name=f"w{b}_v_{ki}")
                    nc.sync.dma_start(out=w, in_=wv_h[ki * P : (ki + 1) * P, :])
                    wv_t.append(w)
                v_sb = []
                for mt in range(NT):
                    vt = sb.tile([P, C], bf16, tag="v", bufs=6, name=f"v{b}_{mt}")
                    for cf0, cfw in CF_SLICES:
                        pm = ps.tile(
                            [P, cfw], f32, tag="mm", bufs=6, name=f"pv{b}_{mt}_{cf0}"
                        )
                        for ki in range(CT):
                            nc.tensor.matmul(
                                pm,
                                xT[ki][:, mt * P : (mt + 1) * P],
                                wv_t[ki][:, cf0 : cf0 + cfw],
                                start=(ki == 0),
                                stop=(ki == CT - 1),
                            )
                        nc.vector.tensor_tensor(
                            vt[:, cf0 : cf0 + cfw],
                            pm,
                            bv_sb[:, cf0 : cf0 + cfw],
                            ALU.add,
                        )
                    v_sb.append(vt)

                # ---- Phase C: scores + masked softmax per n-tile
                attn = []
                for it in range(NT):
                    pm = ps.tile([P, N], f32, tag="mm", bufs=6, name=f"psc{b}_{it}")
                    for ki in range(CT):
                        nc.tensor.matmul(
                            pm,
                            qT[ki][:, it * P : (it + 1) * P],
                            kT[ki],
                            start=(ki == 0),
                            stop=(ki == CT - 1),
                        )
                    mf = sb.tile([P, N], bf16, tag="mf", bufs=3, name=f"mf{b}_{it}")
                    nc.sync.dma_start(out=mf, in_=m_h[b, it * P : (it + 1) * P, :])
                    t = sb.tile([P, N], f32, tag="t", bufs=3, name=f"t{b}_{it}")
                    nc.vector.scalar_tensor_tensor(
                        out=t, in0=pm, scalar=BIG, in1=mf, op0=ALU.add, op1=ALU.mult
                    )
                    mx = sb.tile([P, 1], f32, tag="mx", bufs=2, name=f"mx{b}_{it}")
                    nc.vector.tensor_reduce(
                        out=mx, in_=t, axis=mybir.AxisListType.X, op=ALU.max
                    )
                    bias_ap = sb.tile([P, 1], f32, tag="bias", bufs=2, name=f"ba{b}_{it}")
                    nc.vector.tensor_scalar_mul(out=bias_ap, in0=mx, scalar1=-SCALE)
                    e = sb.tile([P, N], f32, tag="e", bufs=3, name=f"e{b}_{it}")
                    rs = sb.tile([P, 1], f32, tag="rs", bufs=2, name=f"rs{b}_{it}")
                    nc.scalar.activation(
                        out=e, in_=t, func=AF.Exp, bias=bias_ap, scale=SCALE, accum_out=rs
                    )
                    r = sb.tile([P, 1], f32, tag="r", bufs=2, name=f"r{b}_{it}")
                    nc.vector.reciprocal(out=r, in_=rs)
                    at = sb.tile([P, N], bf16, tag="attn", bufs=6, name=f"at{b}_{it}")
                    nc.vector.tensor_scalar_mul(out=at, in0=e, scalar1=r)
                    attn.append(at)

                # ---- Phase E: att_featT[c,n] = sum_m v[m,c] * attn[m,n]
                afT = []
                for co in range(CT):
                    pm = ps.tile([P, N], f32, tag="mm", bufs=6, name=f"pa{b}_{co}")
                    for mt in range(NT):
                        nc.tensor.matmul(
                            pm,
                            v_sb[mt][:, co * P : (co + 1) * P],
                            attn[mt],
                            start=(mt == 0),
                            stop=(mt == NT - 1),
                        )
                    af = sb.tile([P, N], bf16, tag="afT", bufs=12, name=f"af{b}_{co}")
                    nc.vector.tensor_copy(out=af, in_=pm)
                    afT.append(af)

                # ---- Phase F: out = att_feat @ Wr + br
                wr_t = []
                for ki in range(CT):
                    w = sb.tile([P, C], bf16, tag="w", bufs=16, name=f"w{b}_r_{ki}")
                    nc.sync.dma_start(out=w, in_=wr_h[ki * P : (ki + 1) * P, :])
                    wr_t.append(w)
                for it in range(NT):
                    osb = sb.tile([P, C], f32, tag="osb", bufs=3, name=f"o{b}_{it}")
                    for cf0, cfw in CF_SLICES:
                        pm = ps.tile(
                            [P, cfw], f32, tag="mm", bufs=6, name=f"po{b}_{it}_{cf0}"
                        )
                        for co in range(CT):
                            nc.tensor.matmul(
                                pm,
                                afT[co][:, it * P : (it + 1) * P],
                                wr_t[co][:, cf0 : cf0 + cfw],
                                start=(co == 0),
                                stop=(co == CT - 1),
                            )
                        nc.vector.tensor_tensor(
                            osb[:, cf0 : cf0 + cfw],
                            pm,
                            br_sb[:, cf0 : cf0 + cfw],
                            ALU.add,
                        )
                    nc.sync.dma_start(
                        out=out_h[b, it * P : (it + 1) * P, :], in_=osb
                    )
    nc.finalize()
    return nc


def _get_nc():
    if "nc" not in _CACHE:
        _CACHE["nc"] = _build_nc()
    return _CACHE["nc"]


def _run(inputs, trace=False):
    import ml_dtypes
    from concourse import bass_utils

    bf = ml_dtypes.bfloat16
    nc = _get_nc()
    x = np.ascontiguousarray(inputs["x"]).astype(bf)
    mask = np.ascontiguousarray(inputs["Mask"]).astype(bf)
    shared = {
        "wq": np.ascontiguousarray(inputs["Wq"]).astype(bf),
        "bq": np.ascontiguousarray(inputs["bq"], dtype=np.float32),
        "wk": np.ascontiguousarray(inputs["Wk"]).astype(bf),
        "bk": np.ascontiguousarray(inputs["bk"], dtype=np.float32),
        "wv": np.ascontiguousarray(inputs["Wv"]).astype(bf),
        "bv": np.ascontiguousarray(inputs["bv"], dtype=np.float32),
        "wr": np.ascontiguousarray(inputs["Wr"]).astype(bf),
        "br": np.ascontiguousarray(inputs["br"], dtype=np.float32),
    }
    in_maps = [
        {"x": x[c * BPC : (c + 1) * BPC], "mask": mask[c * BPC : (c + 1) * BPC], **shared}
        for c in range(NCORES)
    ]
    res = bass_utils.run_bass_kernel_spmd(
        nc, in_maps, core_ids=list(range(NCORES)), trace=trace
    )
    out = np.concatenate([r["out"] for r in res.results], axis=0)
    return out, res


def kernel(**inputs):
    out, _ = _run(inputs)
    return out



# revision 2
# speedup vs baseline: 1.5850x; 1.5850x over previous
"""Bass/Tile kernel for KeyFrameAttention on 8 NeuronCores (TRN2).

Math (per batch item b):
    q = x @ Wq + bq ; k = x @ Wk + bk ; v = x @ Wv + bv
    scores[n,m] = q[n]·k[m];  masked-fill(Mask==0, -1e20); softmax over m of scores/sqrt(C)
    att_feat[n,c] = sum_m v[m,c] * attn[m,n]          (attention applied TRANSPOSED)
    out = att_feat @ Wr + br
Sharding: data-parallel over batch B=64 -> 8 batch items per core.

The end-to-end time here is dominated by host<->device transfer over the
axon tunnel (~55 MB/s shared), so the kernel is built to minimize wire bytes:
  - x ships as bf16.
  - Mask ships BIT-PACKED (np.packbits along m, 2.1 MB total); unpacked
    on-device with DVE shift/and bitvec ops.
  - The four CxC weights ship as per-core row-shards of hstack(Wq,Wk,Wv,Wr)
    (13.1 MB total instead of 8 full copies = 105 MB) and are AllGathered
    on-device, then kept resident in SBUF for all 8 batch items.
  - Output is fp16 (halves both the donated zero-buffer upload and the
    result download); host casts back to fp32.

Per-core device plan (bf16 matmuls, fp32 PSUM accumulation):
  xT  [C,N]  via DMA-transpose of x tiles
  qT,kT [C,N] = W.T @ x.T   (lhsT = W tile, rhs = xT)      -> bf16 SBUF
  v   [N,C]   natural       (lhsT = xT tile, rhs = Wv)     -> bf16 SBUF
  scores tile [128n, 512m] = qT.T @ kT ; masked softmax via the (+BIG)*mask trick:
      t = (scores + BIG)*mask ; e = exp(s*t - s*max(t)) ; masked -> exp(-s*max) == 0
  att_featT [C,N]: lhsT = v tile, rhs = attn tile (no attn transpose needed)
  out [N,C]:  lhsT = afT tile, rhs = Wr ; + br ; DMA out as fp16.
"""

import math

import numpy as np

B, N, C = 64, 512, 1280
NCORES = 8
BPC = B // NCORES  # batch items per core
P = 128
NT = N // P  # 4  n-tiles
CT = C // P  # 10 c-tiles
W4 = 4 * C  # hstacked weight width
SHARD = C // NCORES  # weight shard rows per core
MPW = N // 8  # packed mask bytes per row
BIG = 10000.0
SCALE = 1.0 / math.sqrt(float(C))
CF_SLICES = [(0, 512), (512, 512), (1024, 256)]  # free-dim chunks of C

_CACHE = {}


def _build_nc():
    import concourse.bass as bass
    import concourse.mybir as mybir
    import concourse.tile as tile
    from concourse import bacc

    f32 = mybir.dt.float32
    f16 = mybir.dt.float16
    bf16 = mybir.dt.bfloat16
    u8 = mybir.dt.uint8
    AF = mybir.ActivationFunctionType
    ALU = mybir.AluOpType

    nc = bacc.Bacc(None, target_bir_lowering=False)
    x_h = nc.declare_dram_parameter("x", [BPC, N, C], bf16, isOutput=False)
    mp_h = nc.declare_dram_parameter("mp", [BPC, N, MPW], u8, isOutput=False)
    ws_h = nc.declare_dram_parameter("ws", [SHARD, W4], bf16, isOutput=False)
    bq_h = nc.declare_dram_parameter("bq", [C], f32, isOutput=False)
    bk_h = nc.declare_dram_parameter("bk", [C], f32, isOutput=False)
    bv_h = nc.declare_dram_parameter("bv", [C], f32, isOutput=False)
    br_h = nc.declare_dram_parameter("br", [C], f32, isOutput=False)
    out_h = nc.declare_dram_parameter("out", [BPC, N, C], f16, isOutput=True)

    ws_int = nc.dram_tensor("ws_int", [SHARD, W4], bf16)
    w_full = nc.dram_tensor("w_full", [C, W4], bf16, addr_space="Shared")

    def bcast_ap(handle):
        ap0 = handle[:]
        return bass.AP(tensor=ap0.tensor, offset=ap0.offset, ap=[[0, P], ap0.ap[0]])

    with tile.TileContext(nc) as tc:
        with (
            tc.tile_pool(name="sb", bufs=1) as sb,
            tc.tile_pool(name="ps", bufs=1, space="PSUM") as ps,
        ):
            # ---- AllGather the weight shards, then park all 4 weights in SBUF
            nc.sync.dma_start(out=ws_int[:], in_=ws_h[:])
            nc.gpsimd.collective_compute(
                "AllGather",
                ALU.bypass,
                replica_groups=[list(range(NCORES))],
                ins=[ws_int[:]],
                outs=[w_full[:]],
            )
            wq_t, wk_t, wv_t, wr_t = [], [], [], []
            for wi, dst in enumerate((wq_t, wk_t, wv_t, wr_t)):
                for ki in range(CT):
                    w = sb.tile([P, C], bf16, tag="w", bufs=4 * CT, name=f"w{wi}_{ki}")
                    nc.sync.dma_start(
                        out=w,
                        in_=w_full[ki * P : (ki + 1) * P, wi * C : (wi + 1) * C],
                    )
                    dst.append(w)

            # ---- biases (one-time)
            bq_sb = sb.tile([P, CT], f32, tag="bq", bufs=1, name="bq_sb")
            nc.sync.dma_start(out=bq_sb, in_=bq_h[:].rearrange("(co p) -> p co", p=P))
            bk_sb = sb.tile([P, CT], f32, tag="bk", bufs=1, name="bk_sb")
            nc.sync.dma_start(out=bk_sb, in_=bk_h[:].rearrange("(co p) -> p co", p=P))
            bv_sb = sb.tile([P, C], f32, tag="bv", bufs=1, name="bv_sb")
            nc.sync.dma_start(out=bv_sb, in_=bcast_ap(bv_h))
            br_sb = sb.tile([P, C], f32, tag="br", bufs=1, name="br_sb")
            nc.sync.dma_start(out=br_sb, in_=bcast_ap(br_h))

            for b in range(BPC):
                # ---- Phase A: DMA-transpose x (bf16) -> xT [c, n]
                xT = []
                for ct in range(CT):
                    xt = sb.tile([P, N], bf16, tag="xT", bufs=14, name=f"xt{b}_{ct}")
                    nc.sync.dma_start_transpose(
                        out=xt, in_=x_h[b, :, ct * P : (ct + 1) * P]
                    )
                    xT.append(xt)

                # ---- Phase B: qT, kT (lhsT = W tile), v (lhsT = xT tile)
                qT, kT = [], []
                for wt, dst, bias, wtag in (
                    (wq_t, qT, bq_sb, "q"),
                    (wk_t, kT, bk_sb, "k"),
                ):
                    for co in range(CT):
                        pm = ps.tile(
                            [P, N], f32, tag="mm", bufs=6, name=f"pq{b}_{wtag}_{co}"
                        )
                        for ki in range(CT):
                            nc.tensor.matmul(
                                pm,
                                wt[ki][:, co * P : (co + 1) * P],
                                xT[ki],
                                start=(ki == 0),
                                stop=(ki == CT - 1),
                            )
                        sbt = sb.tile(
                            [P, N], bf16, tag="qkT", bufs=22, name=f"qk{b}_{wtag}_{co}"
                        )
                        nc.vector.tensor_scalar_add(
                            out=sbt, in0=pm, scalar1=bias[:, co : co + 1]
                        )
                        dst.append(sbt)

                v_sb = []
                for mt in range(NT):
                    vt = sb.tile([P, C], bf16, tag="v", bufs=5, name=f"v{b}_{mt}")
                    for cf0, cfw in CF_SLICES:
                        pm = ps.tile(
                            [P, cfw], f32, tag="mm", bufs=6, name=f"pv{b}_{mt}_{cf0}"
                        )
                        for ki in range(CT):
                            nc.tensor.matmul(
                                pm,
                                xT[ki][:, mt * P : (mt + 1) * P],
                                wv_t[ki][:, cf0 : cf0 + cfw],
                                start=(ki == 0),
                                stop=(ki == CT - 1),
                            )
                        nc.vector.tensor_tensor(
                            vt[:, cf0 : cf0 + cfw],
                            pm,
                            bv_sb[:, cf0 : cf0 + cfw],
                            ALU.add,
                        )
                    v_sb.append(vt)

                # ---- Phase C: scores + masked softmax per n-tile
                attn = []
                for it in range(NT):
                    pm = ps.tile([P, N], f32, tag="mm", bufs=6, name=f"psc{b}_{it}")
                    for ki in range(CT):
                        nc.tensor.matmul(
                            pm,
                            qT[ki][:, it * P : (it + 1) * P],
                            kT[ki],
                            start=(ki == 0),
                            stop=(ki == CT - 1),
                        )
                    # unpack mask bits -> bf16 0/1
                    mpt = sb.tile([P, MPW], u8, tag="mp", bufs=3, name=f"mp{b}_{it}")
                    nc.sync.dma_start(out=mpt, in_=mp_h[b, it * P : (it + 1) * P, :])
                    mu = sb.tile([P, N], u8, tag="mu", bufs=3, name=f"mu{b}_{it}")
                    for j in range(8):
                        nc.vector.tensor_scalar(
                            out=mu[:, j::8],
                            in0=mpt,
                            scalar1=7 - j,
                            scalar2=1,
                            op0=ALU.logical_shift_right,
                            op1=ALU.bitwise_and,
                        )
                    mf = sb.tile([P, N], bf16, tag="mf", bufs=3, name=f"mf{b}_{it}")
                    nc.vector.tensor_copy(out=mf, in_=mu)

                    t = sb.tile([P, N], f32, tag="t", bufs=3, name=f"t{b}_{it}")
                    nc.vector.scalar_tensor_tensor(
                        out=t, in0=pm, scalar=BIG, in1=mf, op0=ALU.add, op1=ALU.mult
                    )
                    mx = sb.tile([P, 1], f32, tag="mx", bufs=2, name=f"mx{b}_{it}")
                    nc.vector.tensor_reduce(
                        out=mx, in_=t, axis=mybir.AxisListType.X, op=ALU.max
                    )
                    bias_ap = sb.tile([P, 1], f32, tag="bias", bufs=2, name=f"ba{b}_{it}")
                    nc.vector.tensor_scalar_mul(out=bias_ap, in0=mx, scalar1=-SCALE)
                    e = sb.tile([P, N], f32, tag="e", bufs=3, name=f"e{b}_{it}")
                    rs = sb.tile([P, 1], f32, tag="rs", bufs=2, name=f"rs{b}_{it}")
                    nc.scalar.activation(
                        out=e, in_=t, func=AF.Exp, bias=bias_ap, scale=SCALE, accum_out=rs
                    )
                    r = sb.tile([P, 1], f32, tag="r", bufs=2, name=f"r{b}_{it}")
                    nc.vector.reciprocal(out=r, in_=rs)
                    at = sb.tile([P, N], bf16, tag="attn", bufs=6, name=f"at{b}_{it}")
                    nc.vector.tensor_scalar_mul(out=at, in0=e, scalar1=r)
                    attn.append(at)

                # ---- Phase E: att_featT[c,n] = sum_m v[m,c] * attn[m,n]
                afT = []
                for co in range(CT):
                    pm = ps.tile([P, N], f32, tag="mm", bufs=6, name=f"pa{b}_{co}")
                    for mt in range(NT):
                        nc.tensor.matmul(
                            pm,
                            v_sb[mt][:, co * P : (co + 1) * P],
                            attn[mt],
                            start=(mt == 0),
                            stop=(mt == NT - 1),
                        )
                    af = sb.tile([P, N], bf16, tag="afT", bufs=12, name=f"af{b}_{co}")
                    nc.vector.tensor_copy(out=af, in_=pm)
                    afT.append(af)

                # ---- Phase F: out = att_feat @ Wr + br
                for it in range(NT):
                    osb = sb.tile([P, C], f16, tag="osb", bufs=3, name=f"o{b}_{it}")
                    for cf0, cfw in CF_SLICES:
                        pm = ps.tile(
                            [P, cfw], f32, tag="mm", bufs=6, name=f"po{b}_{it}_{cf0}"
                        )
                        for co in range(CT):
                            nc.tensor.matmul(
                                pm,
                                afT[co][:, it * P : (it + 1) * P],
                                wr_t[co][:, cf0 : cf0 + cfw],
                                start=(co == 0),
                                stop=(co == CT - 1),
                            )
                        nc.vector.tensor_tensor(
                            osb[:, cf0 : cf0 + cfw],
                            pm,
                            br_sb[:, cf0 : cf0 + cfw],
                            ALU.add,
                        )
                    nc.sync.dma_start(
                        out=out_h[b, it * P : (it + 1) * P, :], in_=osb
                    )
    nc.finalize()
    return nc


def _get_nc():
    if "nc" not in _CACHE:
        _CACHE["nc"] = _build_nc()
    return _CACHE["nc"]


def _run(inputs, trace=False):
    import ml_dtypes
    from concourse import bass_utils

    bf = ml_dtypes.bfloat16
    nc = _get_nc()
    x = np.ascontiguousarray(inputs["x"]).astype(bf)
    mask = np.asarray(inputs["Mask"])
    mp = np.packbits(mask.astype(np.uint8), axis=-1)  # [B, N, N/8]
    wcat = np.concatenate(
        [
            np.asarray(inputs["Wq"]),
            np.asarray(inputs["Wk"]),
            np.asarray(inputs["Wv"]),
            np.asarray(inputs["Wr"]),
        ],
        axis=1,
    ).astype(bf)  # [C, 4C]
    shared = {
        "bq": np.ascontiguousarray(inputs["bq"], dtype=np.float32),
        "bk": np.ascontiguousarray(inputs["bk"], dtype=np.float32),
        "bv": np.ascontiguousarray(inputs["bv"], dtype=np.float32),
        "br": np.ascontiguousarray(inputs["br"], dtype=np.float32),
    }
    in_maps = [
        {
            "x": x[c * BPC : (c + 1) * BPC],
            "mp": mp[c * BPC : (c + 1) * BPC],
            "ws": wcat[c * SHARD : (c + 1) * SHARD],
            **shared,
        }
        for c in range(NCORES)
    ]
    res = bass_utils.run_bass_kernel_spmd(
        nc, in_maps, core_ids=list(range(NCORES)), trace=trace
    )
    out = np.concatenate([r["out"] for r in res.results], axis=0).astype(np.float32)
    return out, res


def kernel(**inputs):
    out, _ = _run(inputs)
    return out


# revision 6
# speedup vs baseline: 2.4256x; 1.5303x over previous
"""Bass/Tile kernel for KeyFrameAttention on 8 NeuronCores (TRN2).

Math (per batch item b):
    q = x @ Wq + bq ; k = x @ Wk + bk ; v = x @ Wv + bv
    scores[n,m] = q[n]·k[m];  masked-fill(Mask==0, -1e20); softmax over m of scores/sqrt(C)
    att_feat[n,c] = sum_m v[m,c] * attn[m,n]          (attention applied TRANSPOSED)
    out = att_feat @ Wr + br
Sharding: data-parallel over batch B=64 -> 8 batch items per core.

The end-to-end time here is dominated by host<->device transfer over the
axon tunnel (~55 MB/s shared), so the kernel is built to minimize wire bytes:
  - x ships as bf16.
  - Mask ships BIT-PACKED (np.packbits along m, 2.1 MB total); unpacked
    on-device with DVE shift/and bitvec ops.
  - The four CxC weights ship as per-core row-shards of hstack(Wq,Wk,Wv,Wr)
    (13.1 MB total instead of 8 full copies = 105 MB) and are AllGathered
    on-device, then kept resident in SBUF for all 8 batch items.
  - Output is fp16 (halves both the donated zero-buffer upload and the
    result download); host casts back to fp32.

Per-core device plan (bf16 matmuls, fp32 PSUM accumulation):
  xT  [C,N]  via DMA-transpose of x tiles
  qT,kT [C,N] = W.T @ x.T   (lhsT = W tile, rhs = xT)      -> bf16 SBUF
  v   [N,C]   natural       (lhsT = xT tile, rhs = Wv)     -> bf16 SBUF
  scores tile [128n, 512m] = qT.T @ kT ; masked softmax via the (+BIG)*mask trick:
      t = (scores + BIG)*mask ; e = exp(s*t - s*max(t)) ; masked -> exp(-s*max) == 0
  att_featT [C,N]: lhsT = v tile, rhs = attn tile (no attn transpose needed)
  out [N,C]:  lhsT = afT tile, rhs = Wr ; + br ; DMA out as fp16.
"""

import math

import numpy as np

B, N, C = 64, 512, 1280
NCORES = 8
BPC = B // NCORES  # batch items per core
P = 128
NT = N // P  # 4  n-tiles
CT = C // P  # 10 c-tiles
W4 = 4 * C  # hstacked weight width
SHARD = C // NCORES  # weight shard rows per core
MPW = N // 8  # packed mask bytes per row
BIG = 10000.0
SCALE = 1.0 / math.sqrt(float(C))
CF_SLICES = [(0, 512), (512, 512), (1024, 256)]  # free-dim chunks of C

_CACHE = {}


def _build_nc():
    import concourse.bass as bass
    import concourse.mybir as mybir
    import concourse.tile as tile
    from concourse import bacc

    f32 = mybir.dt.float32
    f16 = mybir.dt.float16
    bf16 = mybir.dt.bfloat16
    u8 = mybir.dt.uint8
    AF = mybir.ActivationFunctionType
    ALU = mybir.AluOpType

    nc = bacc.Bacc(None, target_bir_lowering=False)
    x_h = nc.declare_dram_parameter("x", [BPC, N, C], bf16, isOutput=False)
    mp_h = nc.declare_dram_parameter("mp", [BPC, N, MPW], u8, isOutput=False)
    ws_h = nc.declare_dram_parameter("ws", [SHARD, W4], bf16, isOutput=False)
    bq_h = nc.declare_dram_parameter("bq", [C], f32, isOutput=False)
    bk_h = nc.declare_dram_parameter("bk", [C], f32, isOutput=False)
    bv_h = nc.declare_dram_parameter("bv", [C], f32, isOutput=False)
    br_h = nc.declare_dram_parameter("br", [C], f32, isOutput=False)
    out_h = nc.declare_dram_parameter("out", [BPC, N, C], f16, isOutput=True)

    ws_int = nc.dram_tensor("ws_int", [SHARD, W4], bf16)
    w_full = nc.dram_tensor("w_full", [C, W4], bf16, addr_space="Shared")

    def bcast_ap(handle):
        ap0 = handle[:]
        return bass.AP(tensor=ap0.tensor, offset=ap0.offset, ap=[[0, P], ap0.ap[0]])

    with tile.TileContext(nc) as tc:
        with (
            tc.tile_pool(name="sb", bufs=1) as sb,
            tc.tile_pool(name="ps", bufs=1, space="PSUM") as ps,
        ):
            # ---- AllGather the weight shards, then park all 4 weights in SBUF
            nc.sync.dma_start(out=ws_int[:], in_=ws_h[:])
            nc.gpsimd.collective_compute(
                "AllGather",
                ALU.bypass,
                replica_groups=[list(range(NCORES))],
                ins=[ws_int[:]],
                outs=[w_full[:]],
            )
            wq_t, wk_t, wv_t, wr_t = [], [], [], []
            for wi, dst in enumerate((wq_t, wk_t, wv_t, wr_t)):
                for ki in range(CT):
                    w = sb.tile([P, C], bf16, tag="w", bufs=4 * CT, name=f"w{wi}_{ki}")
                    nc.sync.dma_start(
                        out=w,
                        in_=w_full[ki * P : (ki + 1) * P, wi * C : (wi + 1) * C],
                    )
                    dst.append(w)

            # ---- biases (one-time)
            bq_sb = sb.tile([P, CT], f32, tag="bq", bufs=1, name="bq_sb")
            nc.sync.dma_start(out=bq_sb, in_=bq_h[:].rearrange("(co p) -> p co", p=P))
            bk_sb = sb.tile([P, CT], f32, tag="bk", bufs=1, name="bk_sb")
            nc.sync.dma_start(out=bk_sb, in_=bk_h[:].rearrange("(co p) -> p co", p=P))
            bv_sb = sb.tile([P, C], f32, tag="bv", bufs=1, name="bv_sb")
            nc.sync.dma_start(out=bv_sb, in_=bcast_ap(bv_h))
            br_sb = sb.tile([P, C], f32, tag="br", bufs=1, name="br_sb")
            nc.sync.dma_start(out=br_sb, in_=bcast_ap(br_h))

            for b in range(BPC):
                # ---- Phase A: DMA-transpose x (bf16) -> xT [c, n]
                xT = []
                for ct in range(CT):
                    xt = sb.tile([P, N], bf16, tag="xT", bufs=14, name=f"xt{b}_{ct}")
                    nc.sync.dma_start_transpose(
                        out=xt, in_=x_h[b, :, ct * P : (ct + 1) * P]
                    )
                    xT.append(xt)

                # ---- Phase B: qT, kT (lhsT = W tile), v (lhsT = xT tile)
                qT, kT = [], []
                for wt, dst, bias, wtag in (
                    (wq_t, qT, bq_sb, "q"),
                    (wk_t, kT, bk_sb, "k"),
                ):
                    for co in range(CT):
                        pm = ps.tile(
                            [P, N], f32, tag="mm", bufs=6, name=f"pq{b}_{wtag}_{co}"
                        )
                        for ki in range(CT):
                            nc.tensor.matmul(
                                pm,
                                wt[ki][:, co * P : (co + 1) * P],
                                xT[ki],
                                start=(ki == 0),
                                stop=(ki == CT - 1),
                            )
                        sbt = sb.tile(
                            [P, N], bf16, tag="qkT", bufs=22, name=f"qk{b}_{wtag}_{co}"
                        )
                        nc.vector.tensor_scalar_add(
                            out=sbt, in0=pm, scalar1=bias[:, co : co + 1]
                        )
                        dst.append(sbt)

                v_sb = []
                for mt in range(NT):
                    vt = sb.tile([P, C], bf16, tag="v", bufs=5, name=f"v{b}_{mt}")
                    for cf0, cfw in CF_SLICES:
                        pm = ps.tile(
                            [P, cfw], f32, tag="mm", bufs=6, name=f"pv{b}_{mt}_{cf0}"
                        )
                        for ki in range(CT):
                            nc.tensor.matmul(
                                pm,
                                xT[ki][:, mt * P : (mt + 1) * P],
                                wv_t[ki][:, cf0 : cf0 + cfw],
                                start=(ki == 0),
                                stop=(ki == CT - 1),
                            )
                        nc.vector.tensor_tensor(
                            vt[:, cf0 : cf0 + cfw],
                            pm,
                            bv_sb[:, cf0 : cf0 + cfw],
                            ALU.add,
                        )
                    v_sb.append(vt)

                # ---- Phase C: scores + masked softmax per n-tile
                attn = []
                for it in range(NT):
                    pm = ps.tile([P, N], f32, tag="mm", bufs=6, name=f"psc{b}_{it}")
                    for ki in range(CT):
                        nc.tensor.matmul(
                            pm,
                            qT[ki][:, it * P : (it + 1) * P],
                            kT[ki],
                            start=(ki == 0),
                            stop=(ki == CT - 1),
                        )
                    # unpack mask bits -> bf16 0/1
                    mpt = sb.tile([P, MPW], u8, tag="mp", bufs=3, name=f"mp{b}_{it}")
                    nc.sync.dma_start(out=mpt, in_=mp_h[b, it * P : (it + 1) * P, :])
                    mu = sb.tile([P, N], u8, tag="mu", bufs=3, name=f"mu{b}_{it}")
                    for j in range(8):
                        nc.vector.tensor_scalar(
                            out=mu[:, j::8],
                            in0=mpt,
                            scalar1=7 - j,
                            scalar2=1,
                            op0=ALU.logical_shift_right,
                            op1=ALU.bitwise_and,
                        )
                    mf = sb.tile([P, N], bf16, tag="mf", bufs=3, name=f"mf{b}_{it}")
                    nc.vector.tensor_copy(out=mf, in_=mu)

                    t = sb.tile([P, N], f32, tag="t", bufs=3, name=f"t{b}_{it}")
                    nc.vector.scalar_tensor_tensor(
                        out=t, in0=pm, scalar=BIG, in1=mf, op0=ALU.add, op1=ALU.mult
                    )
                    mx = sb.tile([P, 1], f32, tag="mx", bufs=2, name=f"mx{b}_{it}")
                    nc.vector.tensor_reduce(
                        out=mx, in_=t, axis=mybir.AxisListType.X, op=ALU.max
                    )
                    bias_ap = sb.tile([P, 1], f32, tag="bias", bufs=2, name=f"ba{b}_{it}")
                    nc.vector.tensor_scalar_mul(out=bias_ap, in0=mx, scalar1=-SCALE)
                    e = sb.tile([P, N], f32, tag="e", bufs=3, name=f"e{b}_{it}")
                    rs = sb.tile([P, 1], f32, tag="rs", bufs=2, name=f"rs{b}_{it}")
                    nc.scalar.activation(
                        out=e, in_=t, func=AF.Exp, bias=bias_ap, scale=SCALE, accum_out=rs
                    )
                    r = sb.tile([P, 1], f32, tag="r", bufs=2, name=f"r{b}_{it}")
                    nc.vector.reciprocal(out=r, in_=rs)
                    at = sb.tile([P, N], bf16, tag="attn", bufs=6, name=f"at{b}_{it}")
                    nc.vector.tensor_scalar_mul(out=at, in0=e, scalar1=r)
                    attn.append(at)

                # ---- Phase E: att_featT[c,n] = sum_m v[m,c] * attn[m,n]
                afT = []
                for co in range(CT):
                    pm = ps.tile([P, N], f32, tag="mm", bufs=6, name=f"pa{b}_{co}")
                    for mt in range(NT):
                        nc.tensor.matmul(
                            pm,
                            v_sb[mt][:, co * P : (co + 1) * P],
                            attn[mt],
                            start=(mt == 0),
                            stop=(mt == NT - 1),
                        )
                    af = sb.tile([P, N], bf16, tag="afT", bufs=12, name=f"af{b}_{co}")
                    nc.vector.tensor_copy(out=af, in_=pm)
                    afT.append(af)

                # ---- Phase F: out = att_feat @ Wr + br
                for it in range(NT):
                    osb = sb.tile([P, C], f16, tag="osb", bufs=3, name=f"o{b}_{it}")
                    for cf0, cfw in CF_SLICES:
                        pm = ps.tile(
                            [P, cfw], f32, tag="mm", bufs=6, name=f"po{b}_{it}_{cf0}"
                        )
                        for co in range(CT):
                            nc.tensor.matmul(
                                pm,
                                afT[co][:, it * P : (it + 1) * P],
                                wr_t[co][:, cf0 : cf0 + cfw],
                                start=(co == 0),
                                stop=(co == CT - 1),
                            )
                        nc.vector.tensor_tensor(
                            osb[:, cf0 : cf0 + cfw],
                            pm,
                            br_sb[:, cf0 : cf0 + cfw],
                            ALU.add,
                        )
                    nc.sync.dma_start(
                        out=out_h[b, it * P : (it + 1) * P, :], in_=osb
                    )
    nc.finalize()
    return nc


def _get_nc():
    if "nc" not in _CACHE:
        _CACHE["nc"] = _build_nc()
    return _CACHE["nc"]


USE_BASS_UTILS_SPMD = False  # flip to route through bass_utils.run_bass_kernel_spmd


def _prep_in_maps(inputs):
    import ml_dtypes

    bf = ml_dtypes.bfloat16
    x = np.ascontiguousarray(inputs["x"]).astype(bf)
    mask = np.asarray(inputs["Mask"])
    mp = np.packbits(mask.astype(np.uint8), axis=-1)  # [B, N, N/8]
    wcat = np.concatenate(
        [
            np.asarray(inputs["Wq"]),
            np.asarray(inputs["Wk"]),
            np.asarray(inputs["Wv"]),
            np.asarray(inputs["Wr"]),
        ],
        axis=1,
    ).astype(bf)  # [C, 4C]
    shared = {
        "bq": np.ascontiguousarray(inputs["bq"], dtype=np.float32),
        "bk": np.ascontiguousarray(inputs["bk"], dtype=np.float32),
        "bv": np.ascontiguousarray(inputs["bv"], dtype=np.float32),
        "br": np.ascontiguousarray(inputs["br"], dtype=np.float32),
    }
    return [
        {
            "x": x[c * BPC : (c + 1) * BPC],
            "mp": mp[c * BPC : (c + 1) * BPC],
            "ws": wcat[c * SHARD : (c + 1) * SHARD],
            **shared,
        }
        for c in range(NCORES)
    ]


def _get_sharded():
    """Cached jit dispatcher for the SPMD bass kernel.

    Same dispatch path as bass_utils.run_bass_kernel_spmd under axon
    (bass2jax _bass_exec_p custom call -> PJRT -> NEFF on cores 0-7), with
    two wall-clock fixes: the donated output buffers are created ON DEVICE
    inside the jit (run_bass_via_pjrt ships host zeros every call), and the
    jit object is cached across calls (run_bass_via_pjrt re-traces per call).
    """
    if "sharded" in _CACHE:
        return _CACHE["sharded"]

    import jax
    import jax.numpy as jnp
    from jax.sharding import Mesh, NamedSharding, PartitionSpec
    from jax.experimental.shard_map import shard_map
    from concourse import bass2jax, mybir

    nc = _get_nc()
    bass2jax.install_neuronx_cc_hook()
    partition_name = nc.partition_id_tensor.name if nc.partition_id_tensor else None
    in_names, out_names, out_avals = [], [], []
    for alloc in nc.m.functions[0].allocations:
        if not isinstance(alloc, mybir.MemoryLocationSet):
            continue
        name = alloc.memorylocations[0].name
        if alloc.kind == "ExternalInput":
            if name != partition_name:
                in_names.append(name)
        elif alloc.kind == "ExternalOutput":
            out_names.append(name)
            out_avals.append(
                jax.core.ShapedArray(tuple(alloc.tensor_shape), mybir.dt.np(alloc.dtype))
            )
    n_params = len(in_names)
    n_outs = len(out_names)
    all_in_names = list(in_names) + out_names
    if partition_name is not None:
        all_in_names.append(partition_name)

    def _body(*args):
        operands = list(args)
        if partition_name is not None:
            operands.append(bass2jax.partition_id_tensor())
        outs = bass2jax._bass_exec_p.bind(
            *operands,
            out_avals=tuple(out_avals),
            in_names=tuple(all_in_names),
            out_names=tuple(out_names),
            lowering_input_output_aliases=(),
            sim_require_finite=True,
            sim_require_nnan=True,
            nc=nc,
        )
        return tuple(outs)

    devices = jax.devices()[:NCORES]
    mesh = Mesh(np.asarray(devices), ("core",))
    sharded = jax.jit(
        shard_map(
            _body,
            mesh=mesh,
            in_specs=(PartitionSpec("core"),) * (n_params + n_outs),
            out_specs=(PartitionSpec("core"),) * n_outs,
            check_rep=False,
        ),
        donate_argnums=tuple(range(n_params, n_params + n_outs)),
        keep_unused=True,
    )
    # On-device zero output buffers (no host->device transfer), rebuilt per
    # call because the main call donates them.
    shardings = tuple(
        NamedSharding(mesh, PartitionSpec("core")) for _ in range(n_outs)
    )
    global_out_shapes = [
        (NCORES * a.shape[0], *a.shape[1:]) for a in out_avals
    ]
    zeros_fn = jax.jit(
        lambda: tuple(
            jnp.zeros(s, a.dtype) for s, a in zip(global_out_shapes, out_avals)
        ),
        out_shardings=shardings,
    )
    _CACHE["sharded"] = (sharded, zeros_fn, in_names, out_names)
    return _CACHE["sharded"]


def _run(inputs, trace=False):
    in_maps = _prep_in_maps(inputs)

    if USE_BASS_UTILS_SPMD:
        from concourse import bass_utils

        nc = _get_nc()
        res = bass_utils.run_bass_kernel_spmd(
            nc, in_maps, core_ids=list(range(NCORES)), trace=trace
        )
        out = np.concatenate([r["out"] for r in res.results], axis=0).astype(np.float32)
        return out, res

    sharded, zeros_fn, in_names, out_names = _get_sharded()
    concat_in = [
        np.concatenate([in_maps[c][name] for c in range(NCORES)], axis=0)
        for name in in_names
    ]
    out_arrs = sharded(*concat_in, *zeros_fn())
    out = np.asarray(out_arrs[out_names.index("out")]).astype(np.float32)

    class _Res:
        exec_time_ns = None
        instructions_and_trace = None

    return out, _Res()


def kernel(**inputs):
    out, _ = _run(inputs)
    return out
